# revision 1
# baseline (speedup 1.0000x reference)
"""Trainium2 Bass kernel for nn_BCErrorCNN (dense_cnn).

Network (per sample, input [17, 9]):
  Conv1D(128, k=3, relu) -> [15, 128]   (position 14 dead: never consumed)
  LocallyConnected1D(128, k=3, relu) -> [13, 128]  (position 12 dead)
  MaxPool1D(2) -> [6, 128]
  LocallyConnected1D(128, k=3, relu) -> [4, 128]
  GlobalAvgPool -> [128]; Dense(100, relu); Dense(1, sigmoid)

Sharding: pure data parallelism, batch 32768 -> 8 cores x 4096.

On-chip layout: activations are [feature(partition), batch(free)], fp32
with float32r matmuls (measured rel err ~1.4e-4 vs the fp32 reference).
The input arrives batch-major, so each 128-sample group is transposed on
the PE (two transposes: feature rows 0..127 and 25..152). Conv patch
windows must sit at 32-aligned partitions (matmul base-partition rule),
so SBUF->SBUF DMAs copy each position's 27 patch rows into 32-aligned
"strips"; strip-tile j holds positions 4j+i at partition 32i, letting 4
conv matmuls run concurrently via PE row tiling. Batch tiles are
software-pipelined with a 1-tile skew (conv of tile i+1 is emitted
before lc1 of tile i) so lc1 never stalls on conv evacuations.
"""

import functools

import numpy as np

# ---- constants (hardcoded per problem spec) --------------------------------
N_CORES = 8
B_FULL = 32768
BC = B_FULL // N_CORES  # per-core batch
NB = 512                # batch tile (columns per matmul)
NT = BC // NB           # batch tiles per core
LIN, CIN, F = 17, 9, 128
FEAT = LIN * CIN        # 153
NPOS = 14               # conv positions actually needed (0..13)
NL1 = 12                # lc1 positions needed (0..11)
NPOOL = 6
NL2 = 4
ND1 = 100


def _build_program(nt=NT, lc1_bias_zero=True, lc2_bias_zero=True):
    import concourse.tile as tile
    from concourse import bacc, mybir
    from concourse.masks import make_identity

    F32 = mybir.dt.float32
    F32R = mybir.dt.float32r
    AF = mybir.ActivationFunctionType
    ALU = mybir.AluOpType

    bc = nt * NB
    nc = bacc.Bacc("TRN2", target_bir_lowering=False, debug=False,
                   num_devices=N_CORES)

    # Flat padded input: [bc*153 + 32] so overlapping window reads at the
    # tail stay in bounds.
    x = nc.dram_tensor("x", [bc * FEAT + 32], F32, kind="ExternalInput").ap()
    wc = nc.dram_tensor("wc", [27, F], F32, kind="ExternalInput").ap()
    w1 = nc.dram_tensor("w1", [NL1, 3, F, F], F32, kind="ExternalInput").ap()
    w2 = nc.dram_tensor("w2", [NL2, 3, F, F], F32, kind="ExternalInput").ap()
    wd1 = nc.dram_tensor("wd1", [F, ND1], F32, kind="ExternalInput").ap()
    wd2 = nc.dram_tensor("wd2", [ND1, 1], F32, kind="ExternalInput").ap()
    cb = nc.dram_tensor("cb", [F, 1], F32, kind="ExternalInput").ap()
    b1 = nc.dram_tensor("b1", [F, NL1], F32, kind="ExternalInput").ap()
    b2 = nc.dram_tensor("b2", [F, NL2], F32, kind="ExternalInput").ap()
    db = nc.dram_tensor("db", [ND1, 1], F32, kind="ExternalInput").ap()
    y = nc.dram_tensor("y", [bc], F32, kind="ExternalOutput").ap()

    with tile.TileContext(nc) as tc:
        with (
            tc.tile_pool(name="const", bufs=1) as cpool,
            tc.tile_pool(name="xg", bufs=2) as xpool,
            tc.tile_pool(name="sg", bufs=2) as spool,
            tc.tile_pool(name="h", bufs=2) as hpool,
            tc.tile_pool(name="m", bufs=2) as mpool,
            tc.tile_pool(name="s2", bufs=2) as s2pool,
            tc.tile_pool(name="s3", bufs=2) as s3pool,
            tc.tile_pool(name="orow", bufs=2) as opool,
            tc.tile_pool(name="psT", bufs=1, space="PSUM") as psT,
            tc.tile_pool(name="psC", bufs=2, space="PSUM") as psC,
            tc.tile_pool(name="psL", bufs=3, space="PSUM") as psL,
        ):
            # ---- identity + first input tile before heavy weight DMAs --
            ident = cpool.tile([128, 128], F32)
            make_identity(nc, ident[:])

            def load_X(it):
                # X[p, g*153+f] = x[(it*512+g*128+p)*153+f]
                X = xpool.tile([128, 4 * FEAT], F32, tag="X", name=f"X{it}")
                src = x[it * 512 * FEAT:it * 512 * FEAT + 1].copy()
                src.ap = src.ap[:0] + [[FEAT, 128], [FEAT * 128, 4],
                                       [1, FEAT]]
                dst = X[:, 0:1].copy()
                dst.ap = dst.ap[:1] + [[FEAT, 4], [1, FEAT]]
                nc.sync.dma_start(dst, src)
                return X

            X_pre = {0: load_X(0)}
            if nt > 1:
                X_pre[1] = load_X(1)

            wc4 = cpool.tile([128, F], F32)
            for i in range(4):
                nc.sync.dma_start(wc4[32 * i:32 * i + 27, :].bitcast(F32R), wc[:].bitcast(F32R))
            def lc_weight_src(w, nl):
                # w[l,k,r,m] -> AP [r(partition), (l k), m]
                src = w[0:1, 0:1, 0:1, 0:1].copy().squeeze()
                src.ap = src.ap[:0] + [[F, F], [F * F, nl * 3], [1, F]]
                return src

            w1t = cpool.tile([128, NL1 * 3 * F], F32)
            w2t = cpool.tile([128, NL2 * 3 * F], F32)

            def load_lc_weights():
                nc.sync.dma_start(w1t[:].bitcast(F32R),
                                  lc_weight_src(w1, NL1).bitcast(F32R))
                nc.sync.dma_start(w2t[:].bitcast(F32R),
                                  lc_weight_src(w2, NL2).bitcast(F32R))

            wd1t = cpool.tile([128, ND1], F32)
            nc.sync.dma_start(wd1t[:].bitcast(F32R), wd1[:].bitcast(F32R))
            wd2t = cpool.tile([ND1, 1], F32)
            nc.sync.dma_start(wd2t[:].bitcast(F32R), wd2[:].bitcast(F32R))
            cbt = cpool.tile([F, 1], F32)
            nc.sync.dma_start(cbt[:], cb[:])
            b1t = cpool.tile([F, NL1], F32)
            nc.sync.dma_start(b1t[:], b1[:])
            b2t = cpool.tile([F, NL2], F32)
            nc.sync.dma_start(b2t[:], b2[:])
            dbt = cpool.tile([ND1, 1], F32)
            nc.sync.dma_start(dbt[:], db[:])

            def r(ap):
                return ap.bitcast(F32R)

            H_tiles = {}
            TAB_tiles = {}
            Zrow = cpool.tile([1, nt * NB], F32, name="Zrow")

            def produce_t(it):
                X = X_pre.pop(it) if it in X_pre else load_X(it)

                # ---- transposes: TA = rows 0..127, TB = rows 25..152 ---
                TA = spool.tile([128, NB], F32, tag="TA", name=f"TA{it}")
                TB = spool.tile([128, NB], F32, tag="TB", name=f"TB{it}")
                for dst_s, off in ((TA, 0), (TB, 25)):
                    pT = psT.tile([128, NB], F32, tag="T")
                    for g in range(4):
                        nc.tensor.transpose(
                            pT[:, g * 128:(g + 1) * 128],
                            X[:, g * FEAT + off:g * FEAT + off + 128],
                            ident[:])
                    nc.vector.tensor_copy(dst_s[:], pT[:])
                TAB_tiles[it] = (TA, TB)

            def produce_c(it):
                TA, TB = TAB_tiles.pop(it)
                # ---- strip build (SBUF->SBUF DMA, partition shifts) ----
                # strip j, partition 32i+r holds feature row 9*(4j+i)+r =
                # patch row r of conv position 4j+i.
                Sg = spool.tile([128, 4 * NB], F32, tag="Sg", name=f"Sg{it}")
                for p in range(NPOS):
                    j, i = p // 4, p % 4
                    if p <= 11:
                        src_s = TA[9 * p:9 * p + 27, :]
                    else:
                        src_s = TB[9 * p - 25:9 * p + 2, :]
                    dma_eng = nc.sync if p % 2 == 0 else nc.scalar
                    dma_eng.dma_start(
                        Sg[32 * i:32 * i + 27,
                           j * NB:(j + 1) * NB].bitcast(F32R),
                        src_s.bitcast(F32R))
                if it == min(1, nt - 1):
                    load_lc_weights()

                # ---- conv: 14 positions, row-tiled 4-way --------------
                H = hpool.tile([128, NPOS * NB], F32, tag="H", name=f"H{it}")
                for a in range(NPOS // 2):
                    p0 = 2 * a
                    pC = psC.tile([128, 1024], F32, tag="C",
                                  name=f"pC{it}_{a}")
                    for d in range(2):
                        p = p0 + d
                        j, i = p // 4, p % 4
                        nc.tensor.matmul(
                            pC[:, d * NB:(d + 1) * NB],
                            r(wc4[32 * i:32 * i + 27, :]),
                            r(Sg[32 * i:32 * i + 27, j * NB:(j + 1) * NB]),
                            start=True, stop=True,
                            tile_position=(32 * i, 0))
                    hdst = H[:, p0 * NB:(p0 + 2) * NB].bitcast(F32R)
                    if a % 2 == 0:
                        nc.scalar.activation(hdst, pC[:], AF.Relu,
                                             bias=cbt[:])
                    else:
                        nc.vector.tensor_scalar(
                            hdst, pC[:], cbt[:], 0.0,
                            op0=ALU.add, op1=ALU.max)
                H_tiles[it] = H

            def consume(it):
                H = H_tiles.pop(it)
                # ---- lc1 (12 positions) + fused maxpool+relu ----------
                # DVE/ACT may read only ONE input from PSUM, so each pool
                # pair is: relu-evac even psum -> E, then max(E, odd psum)
                # (max is associative: max(relu(a), b) == relu(max(a, b))).
                M = mpool.tile([128, NPOOL * NB], F32, tag="M")
                for t in range(NPOOL):
                    pair = []
                    E = spool.tile([128, NB], F32, tag="E", name=f"E{it}_{t}")
                    for d in range(2):
                        l = 2 * t + d
                        ps = psL.tile([128, NB], F32, tag="L")
                        for k in range(3):
                            nc.tensor.matmul(
                                ps[:],
                                r(w1t[:, (l * 3 + k) * F:(l * 3 + k + 1) * F]),
                                r(H[:, (l + k) * NB:(l + k + 1) * NB]),
                                start=(k == 0), stop=(k == 2))
                        pair.append(ps)
                        if d == 0:
                            # evac even psum immediately: frees its psL slot
                            if t % 2 == 0:
                                bias = (0.0 if lc1_bias_zero
                                        else b1t[:, 2 * t:2 * t + 1])
                                nc.scalar.activation(E[:], ps[:], AF.Relu,
                                                     bias=bias)
                            elif lc1_bias_zero:
                                nc.vector.tensor_scalar_max(E[:], ps[:], 0.0)
                            else:
                                nc.vector.tensor_scalar(
                                    E[:], ps[:], b1t[:, 2 * t:2 * t + 1],
                                    0.0, op0=ALU.add, op1=ALU.max)
                    mdst = M[:, t * NB:(t + 1) * NB].bitcast(F32R)
                    if lc1_bias_zero:
                        nc.vector.tensor_tensor(mdst, E[:], pair[1][:],
                                                op=ALU.max)
                    else:
                        nc.vector.scalar_tensor_tensor(
                            mdst, pair[1][:], b1t[:, 2 * t + 1:2 * t + 2],
                            E[:], op0=ALU.add, op1=ALU.max)

                # ---- lc2 (4 positions) --------------------------------
                S2 = s2pool.tile([128, NL2 * NB], F32, tag="S2")
                for l in range(NL2):
                    ps = psL.tile([128, NB], F32, tag="L")
                    for k in range(3):
                        nc.tensor.matmul(
                            ps[:],
                            r(w2t[:, (l * 3 + k) * F:(l * 3 + k + 1) * F]),
                            r(M[:, (l + k) * NB:(l + k + 1) * NB]),
                            start=(k == 0), stop=(k == 2))
                    nc.scalar.activation(
                        S2[:, l * NB:(l + 1) * NB].bitcast(F32R), ps[:],
                        AF.Relu, bias=b2t[:, l:l + 1])

                # ---- mean+dense1 (wd1 pre-scaled by 1/4) + dense2 -----
                pD = psC.tile([128, 1024], F32, tag="C", name=f"pD{it}")
                for l in range(NL2):
                    nc.tensor.matmul(
                        pD[0:ND1, 0:NB], r(wd1t[:]),
                        r(S2[:, l * NB:(l + 1) * NB]),
                        start=(l == 0), stop=(l == NL2 - 1))
                S3 = s3pool.tile([ND1, NB], F32, tag="S3")
                nc.scalar.activation(S3[:].bitcast(F32R), pD[0:ND1, 0:NB],
                                     AF.Relu, bias=dbt[:])
                nc.tensor.matmul(pD[0:1, NB:2 * NB], r(wd2t[:]), r(S3[:]),
                                 start=True, stop=True)
                nc.vector.tensor_copy(Zrow[0:1, it * NB:(it + 1) * NB],
                                      pD[0:1, NB:2 * NB])


            for it in range(nt + 1):
                if it < nt:
                    produce_t(it)
                    produce_c(it)
                if it >= 1:
                    consume(it - 1)

            Zp = opool.tile([nt, NB], F32, tag="o", name="Zp")
            zsrc = Zrow[0:1, 0:1].copy()
            zsrc.ap = zsrc.ap[:1] + [[NB, nt], [1, NB]]
            nc.sync.dma_start(Zp[:], zsrc)
            Osig = opool.tile([nt, NB], F32, tag="o2")
            nc.scalar.activation(Osig[:], Zp[:], AF.Sigmoid)
            ydst = y[0:1].copy()
            ydst.ap = ydst.ap[:0] + [[NB, nt], [1, NB]]
            nc.sync.dma_start(ydst, Osig[:])

    nc.compile()
    return nc


@functools.lru_cache(maxsize=4)
def _get_program(nt, bias_flags):
    return _build_program(nt, bias_flags[0], bias_flags[1])


def _prep_in_maps(inputs, conv_w, conv_b, lc1_w, lc1_b, lc2_w, lc2_b,
                  d1_w, d1_b, d2_w, nt=NT, n_cores=N_CORES):
    bc = nt * NB
    lc1_bias_zero = not np.any(lc1_b[:NL1])
    lc2_bias_zero = not np.any(lc2_b)
    f32 = np.float32
    wc_np = np.ascontiguousarray(conv_w.reshape(27, F), dtype=f32)
    w1_np = np.ascontiguousarray(
        lc1_w[:NL1].reshape(NL1, 3, F, F), dtype=f32)
    w2_np = np.ascontiguousarray(lc2_w.reshape(NL2, 3, F, F), dtype=f32)
    wd1_np = np.ascontiguousarray(d1_w, dtype=f32) * np.float32(0.25)
    wd2_np = np.ascontiguousarray(d2_w.reshape(ND1, 1), dtype=f32)
    cb_np = np.ascontiguousarray(conv_b.reshape(F, 1), dtype=f32)
    b1_np = np.ascontiguousarray(lc1_b[:NL1].T, dtype=f32)
    b2_np = np.ascontiguousarray(lc2_b.T, dtype=f32)
    db_np = np.ascontiguousarray(d1_b.reshape(ND1, 1), dtype=f32)
    shared = dict(wc=wc_np, w1=w1_np, w2=w2_np, wd1=wd1_np, wd2=wd2_np,
                  cb=cb_np, b1=b1_np, b2=b2_np, db=db_np)
    in_maps = []
    for c in range(n_cores):
        shard = inputs[c * bc:(c + 1) * bc].reshape(bc * FEAT)
        xflat = np.empty(bc * FEAT + 32, dtype=f32)
        xflat[:bc * FEAT] = shard
        xflat[bc * FEAT:] = 0.0
        in_maps.append(dict(shared, x=xflat))
    return in_maps, (lc1_bias_zero, lc2_bias_zero)


def kernel(inputs, conv_w, conv_b, lc1_w, lc1_b, lc2_w, lc2_b,
           d1_w, d1_b, d2_w):
    from concourse.bass_utils import run_bass_kernel_spmd

    in_maps, bias_flags = _prep_in_maps(
        inputs, conv_w, conv_b, lc1_w, lc1_b, lc2_w, lc2_b, d1_w, d1_b, d2_w)
    nc = _get_program(NT, bias_flags)
    res = run_bass_kernel_spmd(nc, in_maps, list(range(N_CORES)))
    out = np.concatenate([res.results[c]["y"] for c in range(N_CORES)])
    return out.reshape(B_FULL, 1).astype(np.float32)

